# revision 39
# baseline (speedup 1.0000x reference)
"""Trainium2 Bass kernel for speaker-rate positional-encoding attention.

Math (per batch b):
  rate_q = sigmoid(spk @ Wsq.T + bsq);  rate_k = sigmoid(spk @ Wsk.T + bsk)
  pe(x)[l,d] = sin(rate * pos[l] * 10000^(-d/D) + phase[d]),  phase = 0/pi/2
  x  = query + pe_q                      (NO projection needed -- folded)
  k2 = (keys + pe_k) @ (Wk.T @ Wq) + bk @ Wq        [weight folding]
  scores[t,s] = x[t] . k2[s] (+ c[s] if bq != 0)  == q[t] . k[s]
  vpp = values @ (Wo @ Wv).T  (+ Wo @ bv)           [weight folding]
  E = exp(scores^T / sqrt(D));  out[t] = (sum_s E[s,t] vpp[s,:]) /
      (sqrt(D) * sum_s E[s,t]) + (bo + Wo@bv/sqrt(D))

Weight folding halves the projection FLOPs (q/k/v/o projections collapse
into one S x D x D each on the keys/values side; the T=1024-row query side
needs no matmul at all): 120 -> 69 GFLOP total across 16 batches.

Implementation notes:
 - 8 NeuronCores, data-parallel over batch (2 per core), no collectives.
 - Activations feature-on-partition ("transposed"); host pre-transposes.
 - Matmuls in float32r except Et/vpp (bf16: halves SBUF + FWL weight loads).
 - Softmax skips max-subtraction (scores bounded ~ +-15); normalization is
   deferred to a per-partition scalar multiply after E @ vpp.
 - Sin range reduction via f32->i32 round-to-nearest (as the DVE has no
   float mod ISA op): r = ang - 2pi*rint(ang/2pi).  Skipped for d-tiles
   6..7 where rate<1 bounds the angle below pi.
 - Engine/queue plan: sync=weights+consts DMA, gpsimd=input DMA + posenc
   adds, vector=angle math + final scale + out DMA, scalar=sin/drains/exp.
"""

import sys

for _p in ("/opt/trn_rl_repo",):
    if _p not in sys.path:
        sys.path.insert(0, _p)

import numpy as np

import concourse.bass as bass
from concourse import bacc
import concourse.mybir as mybir
import concourse.tile as tile
from concourse.bass_utils import run_bass_kernel_spmd

B, T, S, D, SPK = 16, 1024, 512, 1024, 256
NCORES = 8
BL = B // NCORES          # batches per core
P = 128
DT = D // P               # 8 d-tiles (contraction chunks / m-tiles)
ST = S // P               # 4 s-tiles
TT = T // P               # 8 t-tiles
F32 = mybir.dt.float32
BF16 = mybir.dt.bfloat16
MMDT = mybir.dt.float32r  # matmul compute dtype (full-rate fp32)
SQRT_D = float(np.sqrt(D))
PI = float(np.pi)
TWO_PI = 2.0 * PI

AF = mybir.ActivationFunctionType
ALU = mybir.AluOpType


def build_nc(use_c=False, cconst=0.0, red_q=tuple(range(DT)), red_k=tuple(range(DT))):
    nc = bacc.Bacc()
    dp = nc.declare_dram_parameter
    qTi = dp("qTi", [BL, DT, P, T], MMDT, isOutput=False)   # query^T  [b,dt,p_d,t]
    kTi = dp("kTi", [BL, DT, P, S], BF16, isOutput=False)   # keys^T
    vTi = dp("vTi", [BL, DT, P, S], BF16, isOutput=False)   # values^T
    wkq = dp("wkq", [DT, P, D], BF16, isOutput=False)       # (Wk.T@Wq)[d, m]
    wvo = dp("wvo", [DT, P, D], BF16, isOutput=False)       # (Wo@Wv).T[d, e]
    posq = dp("posq", [T], F32, isOutput=False)             # current_mel_pos + t
    posk = dp("posk", [S], F32, isOutput=False)
    biasf = dp("biasf", [D], F32, isOutput=False)           # sqrt(D)*bo + Wo@bv
    # all small per-partition constants packed into one DMA:
    # invd | phs | phs2p | bkq | spk | wsq | wsk | bsq | bsk
    cblk = dp("cblk", [P, 550], F32, isOutput=False)
    ucd = dp("ucd", [P, DT], BF16, isOutput=False)          # bq @ Wk (c-term)
    out = dp("out", [BL, TT, P, D], F32, isOutput=True)

    def bcast(ap, n=P):
        # replicate a DRAM vector across n partitions (DMA partition-step 0)
        return bass.AP(tensor=ap.tensor, offset=ap.offset, ap=[[0, n], *ap.ap])

    with tile.TileContext(nc) as tc:
        with (
            tc.tile_pool(name="consts", bufs=1) as cp,
            tc.tile_pool(name="tmp", bufs=4) as tp,       # posenc temps
            tc.tile_pool(name="big", bufs=1) as bp,       # long-lived per-batch acts
            tc.tile_pool(name="outp", bufs=2) as op,
            tc.tile_pool(name="psum", bufs=7 if use_c else 8, space="PSUM") as pp,
        ):
            # ---------------- constants ----------------
            # One packed DMA for the small consts; weights in 2-dt pieces so
            # batch 0's dt-outer phases stream as the pieces land. sync ring:
            # cblk -> posk -> wvo; scalar ring: wkq -> posq -> biasf.
            cblk_sb = cp.tile([P, 550], F32)
            nc.sync.dma_start(out=cblk_sb, in_=cblk[:])
            invd_sb = cblk_sb[:, 0:8]
            phs_sb = cblk_sb[:, 8:16]
            phs2p_sb = cblk_sb[:, 16:24]
            bkq_sb = cblk_sb[:, 24:32]
            spk_sb = cblk_sb[:, 32:36]
            wsq_sb = cblk_sb[:, 36:292]
            wsk_sb = cblk_sb[:, 292:548]
            bsq_sb = cblk_sb[:, 548:549]
            bsk_sb = cblk_sb[:, 549:550]
            if use_c:
                uc_sb = cp.tile([P, DT], BF16)
                nc.sync.dma_start(out=uc_sb, in_=ucd[:])
            posk_bc = cp.tile([P, S], F32)
            nc.sync.dma_start(out=posk_bc, in_=bcast(posk[:]))
            wvo_sb = cp.tile([P, DT, D], BF16)
            for c4 in range(4):
                nc.sync.dma_start(
                    out=wvo_sb[:, 2 * c4:2 * c4 + 2],
                    in_=wvo[2 * c4:2 * c4 + 2].rearrange("a b c -> b a c"))
            wkq_sb = cp.tile([P, DT, D], BF16)
            for c4 in range(4):
                nc.scalar.dma_start(
                    out=wkq_sb[:, 2 * c4:2 * c4 + 2],
                    in_=wkq[2 * c4:2 * c4 + 2].rearrange("a b c -> b a c"))
            posq_bc = cp.tile([P, T], F32)
            nc.scalar.dma_start(out=posq_bc, in_=bcast(posq[:]))
            biasf_bc = cp.tile([P, D], F32)   # sqrt(D)*biasf, folded into vpp
            nc.scalar.dma_start(out=biasf_bc, in_=bcast(biasf[:]))

            zero_b = cp.tile([P, 1], F32)
            nc.vector.memset(zero_b, 0.0)
            ones_bf = cp.tile([P, 1], BF16)
            nc.vector.memset(ones_bf, SQRT_D)   # 32.0, exact in bf16

            # ---------------- speaker rates ----------------
            # psum[p, b] = sum_d Ws[d] * spk[b, d]   (replicated over p)
            rate_sb = cp.tile([P, 2 * BL], F32)   # [:, 0:BL]=rate_q, [:, BL:]=rate_k
            for which, wmat, bias in ((0, wsq_sb, bsq_sb), (1, wsk_sb, bsk_sb)):
                ps = pp.tile([P, 512], F32, tag="ps")
                for kt in range(2):
                    nc.tensor.matmul(
                        ps[:, :BL],
                        wmat[:, kt * P:(kt + 1) * P],
                        spk_sb[:, kt * BL:(kt + 1) * BL],
                        start=(kt == 0),
                        stop=(kt == 1),
                    )
                nc.scalar.activation(
                    rate_sb[:, which * BL:(which + 1) * BL], ps[:, :BL],
                    AF.Sigmoid, bias=bias, scale=1.0,
                )
            # srate[p, b, dt] = rate_b * invdiv[p, dt]
            srq = cp.tile([P, BL, DT], F32)
            srk = cp.tile([P, BL, DT], F32)
            srq2 = cp.tile([P, BL, DT], F32)   # srq / 2pi (for the rint cast)
            srk2 = cp.tile([P, BL, DT], F32)
            for b in range(BL):
                nc.vector.tensor_scalar_mul(srq[:, b], invd_sb, rate_sb[:, b:b + 1])
                nc.vector.tensor_scalar_mul(
                    srk[:, b], invd_sb, rate_sb[:, BL + b:BL + b + 1])
                nc.vector.tensor_scalar_mul(srq2[:, b], srq[:, b], 1.0 / TWO_PI)
                nc.vector.tensor_scalar_mul(srk2[:, b], srk[:, b], 1.0 / TWO_PI)

            # ---------------- per-batch pipeline ----------------
            for b in range(BL):
                xT = bp.tile([P, DT, T], MMDT, tag="xT")     # (query+pe)^T [p_m, mt, t]
                k2T = bp.tile([P, DT, S], MMDT, tag="k2T")   # k2^T [p_m, mt, s]
                vpp = bp.tile([P, ST, D], BF16, tag="vpp")   # vpp  [p_s, st, e]
                Et = bp.tile([P, ST, T], BF16, tag="Et")     # exp(scores^T) [p_s, st, t]
                recip = bp.tile([P, TT], F32, tag="recip")

                # ---- input DMAs (gpsimd queue, consumption order) ----
                # ---- input DMAs on the gpsimd ring: one descriptor per
                # tensor (DMA issue costs ~0.65us of engine time each).
                vT = bp.tile([P, DT, S], BF16, tag="vT")
                nc.gpsimd.dma_start(
                    out=vT, in_=vTi[b].rearrange("a b c -> b a c"))
                xkT = bp.tile([P, DT, S], BF16, tag="xkT")
                nc.gpsimd.dma_start(
                    out=xkT, in_=kTi[b].rearrange("a b c -> b a c"))
                for h in range(2):
                    nc.gpsimd.dma_start(
                        out=xT[:, 4 * h:4 * h + 4],
                        in_=qTi[b, 4 * h:4 * h + 4].rearrange("a b c -> b a c"))

                # posenc chunk: dst[:, :n] += sin(pos*sr + phase).  The ACT
                # engine computes the angle itself via per-partition
                # scale/bias, so no DVE angle pass is needed:
                #   reduced:  yi  = rint(pos*sr2p + phs2p)          (scalar)
                #             res = pos*sr2p - yi                   (vector)
                #             pe  = sin(2pi*res + phase)            (scalar)
                #   direct:   pe  = sin(pos*sr + phase)             (scalar)
                def pe_add(dst, pos_bc_sl, b_, dt, n, reduce, tag, nm):
                    sr = (srk, srq)[tag == "q"][:, b_, dt:dt + 1]
                    sr2 = (srk2, srq2)[tag == "q"][:, b_, dt:dt + 1]
                    pe_t = tp.tile([P, n], F32, tag=f"pe{tag}", bufs=2,
                                   name=f"pe{nm}")
                    if reduce:
                        yi = tp.tile([P, n], mybir.dt.int32, tag=f"yi{tag}",
                                     bufs=2, name=f"yi{nm}")
                        nc.scalar.activation(
                            yi, pos_bc_sl, AF.Identity,
                            bias=phs2p_sb[:, dt:dt + 1], scale=sr2)
                        nc.vector.scalar_tensor_tensor(
                            pe_t, pos_bc_sl, sr2, yi, ALU.mult, ALU.subtract)
                        nc.scalar.activation(
                            pe_t, pe_t, AF.Sin,
                            bias=phs_sb[:, dt:dt + 1], scale=TWO_PI)
                    else:
                        nc.scalar.activation(
                            pe_t, pos_bc_sl, AF.Sin,
                            bias=phs_sb[:, dt:dt + 1], scale=sr)
                    nc.vector.tensor_add(dst, dst, pe_t)

                # ---- VPP interleaved with keys posenc.
                # Batch 0 streams dt-outer (consumes wvo/vch chunks as they
                # land on the rings); batch 1 runs bank-at-a-time so the psV
                # drains spread out and pipeline with the k-chunk work.
                psV = [pp.tile([P, 512], F32, tag="ps", name=f"psV{b}_{i}")
                       for i in range(2 * ST)]

                def vpp_mm(i, dt):
                    st, ec = divmod(i, 2)
                    nc.tensor.matmul(
                        psV[i],
                        vT[:, dt, st * P:(st + 1) * P],
                        wvo_sb[:, dt, ec * 512:(ec + 1) * 512],
                        start=(dt == 0),
                        stop=(dt == DT - 1),
                    )

                def vpp_drain(i):
                    # vpp' = psV + sqrt(D)*biasf: with out = recip*(Et^T@vpp')
                    # this reproduces the +biasf term exactly (sum_s Et*c =
                    # c*SumE and recip = 1/(sqrt(D)*SumE)).
                    st, ec = divmod(i, 2)
                    sl = slice(ec * 512, (ec + 1) * 512)
                    nc.vector.tensor_add(vpp[:, st, sl], psV[i], biasf_bc[:, sl])

                if b == 0:
                    for dt in range(DT):
                        for i in range(2 * ST):
                            vpp_mm(i, dt)
                        pe_add(xkT[:, dt], posk_bc, b, dt, S,
                               dt in red_k, "k", f"k{b}_{dt}")
                    for i in range(2 * ST):
                        vpp_drain(i)
                else:
                    for i in range(2 * ST):
                        for dt in range(DT):
                            vpp_mm(i, dt)
                        pe_add(xkT[:, i], posk_bc, b, i, S,
                               i in red_k, "k", f"k{b}_{i}")
                        vpp_drain(i)

                # ---- K2 over mt; batch 0 dt-outer (streams wkq chunks),
                # batch 1 bank-at-a-time (weights resident by then).
                psK = [pp.tile([P, 512], F32, tag="ps", name=f"psK{b}_{i}")
                       for i in range(DT)]
                if use_c:
                    psC = pp.tile([P, 512], F32, tag="psc", name=f"psC{b}", bufs=1)

                def k2_mm(mt, dt):
                    nc.tensor.matmul(
                        psK[mt][:, :S],
                        wkq_sb[:, dt, mt * P:(mt + 1) * P],
                        xkT[:, dt],
                        start=(dt == 0),
                        stop=(dt == DT - 1),
                    )

                if b == 0:
                    for dt in range(DT):
                        for mt in range(DT):
                            k2_mm(mt, dt)
                else:
                    for mt in range(DT):
                        for dt in range(DT):
                            k2_mm(mt, dt)
                if use_c:
                    for dt in range(DT):
                        # c[s] += u[d-chunk] . xk[d-chunk, s], in [p_s, st] layout
                        for st in range(ST):
                            nc.tensor.matmul(
                                psC[:, st:st + 1],
                                xkT[:, dt, st * P:(st + 1) * P],
                                uc_sb[:, dt:dt + 1],
                                start=(dt == 0),
                                stop=(dt == DT - 1),
                                skip_group_check=True,
                            )

                # ---- query posenc (1024-wide chunks)
                for dt in range(DT):
                    pe_add(xT[:, dt], posq_bc, b, dt, T,
                           dt in red_q, "q", f"q{b}_{dt}")

                # k2 drains on scalar, after the q-sins in queue order
                for mt in range(DT):
                    nc.scalar.activation(
                        k2T[:, mt], psK[mt][:, :S], AF.Identity,
                        bias=bkq_sb[:, mt:mt + 1], scale=1.0)
                if use_c:
                    # cb[s, st] = (c + bq.bk) / sqrt(D), used as exp bias
                    cb_sb = cp.tile([P, ST], F32, name=f"cb{b}", tag=f"cb{b}")
                    nc.vector.tensor_scalar(
                        cb_sb, psC[:, :ST], cconst, 1.0 / SQRT_D,
                        ALU.add, ALU.mult)

                # ---- scores, mt-outer: xT chunks are consumed incrementally
                # so the query posenc pipeline has until ~mt's turn, not the
                # phase start, to deliver chunk mt.
                psS = [pp.tile([P, 512], F32, tag="ps", name=f"psS{b}_{i}")
                       for i in range(2 * ST)]
                for mt in range(DT):
                    for i in range(2 * ST):
                        tc2, st = divmod(i, ST)
                        nc.tensor.matmul(
                            psS[i],
                            k2T[:, mt, st * P:(st + 1) * P],
                            xT[:, mt, tc2 * 512:(tc2 + 1) * 512],
                            start=(mt == 0),
                            stop=(mt == DT - 1),
                        )
                for i in range(2 * ST):
                    tc2, st = divmod(i, ST)
                    sl = slice(tc2 * 512, (tc2 + 1) * 512)
                    nc.scalar.activation(
                        Et[:, st, sl], psS[i], AF.Exp,
                        bias=cb_sb[:, st:st + 1] if use_c else zero_b,
                        scale=1.0 / SQRT_D)

                # ---- recip[t] = 1 / (sqrt(D) * sum_s E[s,t])
                for tt in range(TT):
                    ps1 = pp.tile([P, 512], F32, tag="ps", name=f"ps1{b}_{tt}")
                    for st in range(ST):
                        nc.tensor.matmul(
                            ps1[:, 0:1],
                            Et[:, st, tt * P:(tt + 1) * P],
                            ones_bf,
                            start=(st == 0),
                            stop=(st == ST - 1),
                        )
                    nc.vector.reciprocal(recip[:, tt:tt + 1], ps1[:, 0:1])

                # ---- out[t, e] = recip[t] * sum_s E[s,t] vpp'[s,e]
                # (biasf folded into vpp'); the two halves drain on vector
                # and scalar to balance the queues.
                for tt in range(TT):
                    osb = op.tile([P, D], F32, tag="osb", name=f"osb{b}_{tt}")
                    for ec in range(2):
                        sl = slice(ec * 512, (ec + 1) * 512)
                        psF = pp.tile([P, 512], F32, tag="ps",
                                      name=f"psF{b}_{tt}_{ec}")
                        for st in range(ST):
                            nc.tensor.matmul(
                                psF,
                                Et[:, st, tt * P:(tt + 1) * P],
                                vpp[:, st, ec * 512:(ec + 1) * 512],
                                start=(st == 0),
                                stop=(st == ST - 1),
                            )
                        if ec == 0:
                            nc.vector.tensor_scalar_mul(
                                osb[:, sl], psF, recip[:, tt:tt + 1])
                        else:
                            nc.scalar.activation(
                                osb[:, sl], psF, AF.Identity,
                                bias=zero_b, scale=recip[:, tt:tt + 1])
                        nc.sync.dma_start(out=out[b, tt][:, sl], in_=osb[:, sl])
    return nc


def marshal_inputs(query, keys, values, speaker_embedding, Wsq, bsq, Wsk, bsk,
                   Wq, bq, Wk, bk, Wv, bv, Wo, bo, current_mel_pos):
    f = lambda x: np.ascontiguousarray(np.asarray(x, dtype=np.float32))
    query, keys, values = f(query), f(keys), f(values)
    spk = f(speaker_embedding)
    Wsq, Wsk = f(Wsq), f(Wsk)
    Wq, Wk, Wv, Wo = f(Wq), f(Wk), f(Wv), f(Wo)
    bq, bk, bv, bo = f(bq), f(bk), f(bv), f(bo)
    bsq, bsk = f(bsq), f(bsk)
    mel0 = int(np.asarray(current_mel_pos).item())

    dvec = np.arange(D, dtype=np.float32)
    invdiv = (10000.0 ** (-dvec / D)).astype(np.float32)
    phase = np.where(dvec.astype(np.int64) % 2 == 0, 0.0, np.pi / 2).astype(np.float32)

    # d-tiles whose angle bound rate*pos_max*invdiv + pi/2 stays under pi
    # never need range reduction (rate = sigmoid(.) < 1).
    def red_set(pos_max):
        return tuple(dt for dt in range(DT)
                     if pos_max * invdiv[dt * P] + np.pi / 2 > np.pi - 0.01)
    red_k = red_set(float(S - 1))
    red_q = red_set(float(mel0 + T - 1))

    # weight folding (host: weights only, no activation math)
    import ml_dtypes
    BF = ml_dtypes.bfloat16
    Wkq = (Wk.T @ Wq).astype(np.float32)            # k2 = xk @ Wkq
    Wvo = (Wo @ Wv).astype(np.float32)              # vpp = values @ Wvo.T
    bkq_v = (bk @ Wq).astype(np.float32)
    # sqrt(D)*(bo + Wo@bv/sqrt(D)): added to vpp so that
    # recip * (Et^T @ vpp') lands on out + bias exactly.
    biasf_v = (SQRT_D * bo + (Wo @ bv)).astype(np.float32)
    uc_v = (bq @ Wk).astype(np.float32)
    cconst = float(bq @ bk)
    use_c = bool(np.any(bq))

    col = lambda v: np.ascontiguousarray(v.reshape(DT, P).T)
    wsq_rep = np.ascontiguousarray(
        np.repeat(Wsq.reshape(2, P, 1), P, axis=2).transpose(1, 0, 2).reshape(P, 2 * P))
    wsk_rep = np.ascontiguousarray(
        np.repeat(Wsk.reshape(2, P, 1), P, axis=2).transpose(1, 0, 2).reshape(P, 2 * P))
    cblk_base = np.concatenate([
        col(invdiv), col(phase), col((phase / TWO_PI).astype(np.float32)),
        col(bkq_v),
        np.zeros((P, 2 * BL), np.float32),  # spk slot, filled per core
        wsq_rep, wsk_rep,
        np.full((P, 1), bsq.reshape(-1)[0], dtype=np.float32),
        np.full((P, 1), bsk.reshape(-1)[0], dtype=np.float32),
    ], axis=1).astype(np.float32)
    assert cblk_base.shape == (P, 550)
    shared = {
        "wkq": np.ascontiguousarray(Wkq.reshape(DT, P, D)).astype(BF),
        "wvo": np.ascontiguousarray(Wvo.T.reshape(DT, P, D)).astype(BF),
        "posq": (np.arange(T, dtype=np.float32) + mel0),
        "posk": np.arange(S, dtype=np.float32),
        "biasf": biasf_v,
        "ucd": col(uc_v).astype(BF),
    }
    tr = lambda x, L: np.ascontiguousarray(
        x.reshape(BL, L, DT, P).transpose(0, 2, 3, 1))
    in_maps = []
    for c in range(NCORES):
        sl = slice(c * BL, (c + 1) * BL)
        m = dict(shared)
        m["qTi"] = tr(query[sl], T)
        m["kTi"] = tr(keys[sl], S).astype(BF)
        m["vTi"] = tr(values[sl], S).astype(BF)
        cb = cblk_base.copy()
        cb[:, 32:36] = np.ascontiguousarray(
            spk[sl].T.reshape(2, P, BL).transpose(1, 0, 2).reshape(P, 2 * BL))
        m["cblk"] = cb
        in_maps.append(m)
    build_args = dict(use_c=use_c, cconst=cconst, red_q=red_q, red_k=red_k)
    return in_maps, build_args


def run_device(in_maps, build_args=None, trace=False, **kw):
    nc = build_nc(**(build_args or {}))
    if not nc.is_finalized():
        nc.finalize()
    res = run_bass_kernel_spmd(nc, in_maps, core_ids=list(range(NCORES)),
                               trace=trace, **kw)
    outs = [np.asarray(r["out"], dtype=np.float32).reshape(BL, T, D)
            for r in res.results]
    return np.concatenate(outs, axis=0), res


def kernel(**inputs) -> np.ndarray:
    in_maps, build_args = marshal_inputs(**inputs)
    out, _ = run_device(in_maps, build_args)
    return out
